# revision 21
# baseline (speedup 1.0000x reference)
"""Trainium2 Bass kernel for nn_NewModel_42356967473589 (dense_transformer).

Model: two BiAttention blocks + final linear mapping.
  o = BiAttn(ctx, q1) ; o = BiAttn(o, q2) ; out = o @ w_map.T + b_map

Sharding: 8 cores = (batch b in 0..3) x (context half h in 0..1).
Each core owns 1024 context rows of one batch. All compute is row-local
except the softmax-over-context (weight_two); its (sum-exp, weighted-sum)
stats are combined across the pair of cores sharing a batch via a tiny
pairwise AllReduce, overlapped with the large matmuls.

Math restructure (per stage, X = stage input [C,D], M = memory [Q,D]):
  out = X@W1 + o1@W2 + (X*o1)@W3 + (t*o1)@W4      (W_k = w_out[:, kD:(k+1)D].T)
  o1 = P@M (rank Q=64), t broadcast over rows =>
  o1@W2 + (t*o1)@W4 = P @ (M @ (W2 + t*W4))        (rank-64 path)
All tensors are kept transposed on-chip ([D on partitions, rows free]) and
in bf16 (rel tolerance is 2e-2; bf16 end-to-end lands ~5e-3), which halves
HBM traffic, halves LDWEIGHTS (fast weight load), and halves DVE time.
PSUM accumulation stays fp32.

Engine-queue discipline (head-of-line blocking is real):
  sync   = all bulk weight/input DMA (xt, W1/W3, W2/W4, w_map), out stores
  scalar = tiny consts, exp activations, PSUM->SBUF copies, bias adds
  gpsimd = partition max-reduce, collective staging + AllReduce (nothing else)
"""

import numpy as np
import ml_dtypes

import concourse.bacc as bacc
import concourse.tile as tile
from concourse import mybir
from concourse.bass_utils import run_bass_kernel_spmd
from contextlib import ExitStack
import bass_rust

f32 = mybir.dt.float32
bf16 = mybir.dt.bfloat16
i32 = mybir.dt.int32
Alu = mybir.AluOpType
AF = bass_rust.ActivationFunctionType
AX = bass_rust.AxisListType
RedOp = bass_rust.ReduceOp

B, C_LEN, Q_LEN, D = 4, 2048, 64, 1024
N_CORES = 8
R = C_LEN // 2          # rows per core
NK = D // 128           # contraction chunks
RH = R // 512           # row halves (moving-dim tiles)
D2 = 2 * D
NEGBIG = 10000.0
# correction fold threshold per stage: groups j >= FOLD close after the
# collective result lands anyway, so the rank-64 term accumulates in-group;
# earlier groups close immediately and get a separate correction pass.
FOLD = {0: 4, 1: 2}

_CACHED_NC = None


def _build_nc():
    nc = bacc.Bacc("TRN2", target_bir_lowering=False, debug=False,
                   num_devices=N_CORES)

    # ---- per-core DRAM I/O (host pre-tiled layouts, see _shard_inputs) ----
    xt_ap = nc.dram_tensor("xt", [128, NK * R], bf16, kind="ExternalInput").ap()
    m_t = [nc.dram_tensor(f"m{s}t", [128, NK * Q_LEN], bf16, kind="ExternalInput").ap() for s in (1, 2)]
    m_n = [nc.dram_tensor(f"m{s}n", [Q_LEN, D], bf16, kind="ExternalInput").ap() for s in (1, 2)]
    vec = [nc.dram_tensor(f"vec{s}", [128, NK * 3], bf16, kind="ExternalInput").ap() for s in (1, 2)]
    msk = [nc.dram_tensor(f"mask{s}", [Q_LEN, 1], i32, kind="ExternalInput").ap() for s in (1, 2)]
    w1t = [nc.dram_tensor(f"w1t{s}", [D, D], bf16, kind="ExternalInput").ap() for s in (1, 2)]
    w3t = [nc.dram_tensor(f"w3t{s}", [D, D], bf16, kind="ExternalInput").ap() for s in (1, 2)]
    w2c = [nc.dram_tensor(f"w2c{s}", [D, D], bf16, kind="ExternalInput").ap() for s in (1, 2)]
    w4c = [nc.dram_tensor(f"w4c{s}", [D, D], bf16, kind="ExternalInput").ap() for s in (1, 2)]
    wmt_ap = nc.dram_tensor("wmt", [D2, D], bf16, kind="ExternalInput").ap()
    bmap_ap = nc.dram_tensor("bmap", [128, 16], f32, kind="ExternalInput").ap()
    out_ap = nc.dram_tensor("out", [D2, R], bf16, kind="ExternalOutput").ap()

    with tile.TileContext(nc) as tc, ExitStack() as ctx:
        sb_x = ctx.enter_context(tc.tile_pool(name="sb_x", bufs=2))
        sb_xo = ctx.enter_context(tc.tile_pool(name="sb_xo", bufs=1))
        sb_w13 = ctx.enter_context(tc.tile_pool(name="sb_w13", bufs=32))
        sb_ws = ctx.enter_context(tc.tile_pool(name="sb_ws", bufs=8))
        sb_o = ctx.enter_context(tc.tile_pool(name="sb_o", bufs=3))
        sb_rh = ctx.enter_context(tc.tile_pool(name="sb_rh", bufs=2))
        sb_st = ctx.enter_context(tc.tile_pool(name="sb_st", bufs=1))
        ps_o = ctx.enter_context(tc.tile_pool(name="ps_o", bufs=4, space="PSUM"))
        ps_att = ctx.enter_context(tc.tile_pool(name="ps_att", bufs=2, space="PSUM"))
        ps_m = ctx.enter_context(tc.tile_pool(name="ps_m", bufs=2, space="PSUM"))
        dram = ctx.enter_context(tc.tile_pool(name="dram", bufs=2, space="DRAM"))

        # ---- constants ----
        ones_row = sb_st.tile([1, 128], bf16, tag="ones_row")
        nc.vector.memset(ones_row[:], 1.0)
        ones_q = sb_st.tile([Q_LEN, 1], bf16, tag="ones_q")
        nc.vector.memset(ones_q[:], 1.0)

        # ---- PE warmup: ~3.5us of dummy matmuls so the HAM clock-gate
        # opens to 8/8 while the input DMA is still in flight ----
        if True:  # dummy PE warmup: opens the HAM clock gate early
            dum = sb_st.tile([1, 512], bf16, tag="dum")
            nc.vector.memset(dum[:], 0.0)
            ps_w = ps_m.tile([128, 512], f32, tag="ps_m")
            for k in range(36):
                nc.tensor.matmul(ps_w[:], ones_row[:], dum[:],
                                 start=(k == 0), stop=(k == 35))

        # ---- warmup collective: absorbs core start-skew so the stage-1
        # stats AllReduce is fast; gpsimd has nothing urgent until ~+12us ----
        wsb = sb_st.tile([1, 4], f32, tag="wsb")
        nc.vector.memset(wsb[:], 0.0)
        win = dram.tile([1, 4], f32, tag="win")
        wout = dram.tile([1, 4], f32, tag="wout")
        nc.gpsimd.dma_start(win[:], wsb[:])
        nc.gpsimd.collective_compute(
            "AllReduce", Alu.add,
            replica_groups=[[0, 1], [2, 3], [4, 5], [6, 7]],
            ins=[win[:].opt()], outs=[wout[:].opt()])

        # ---- input X^T (sync queue, first) ----
        xt0 = sb_x.tile([128, NK, R], bf16, tag="xt")
        nc.sync.dma_start(xt0[:], xt_ap[:].rearrange("p (c r) -> p c r", c=NK))

        # ---- bulk weight prefetch (sync queue, in consumption order) ----
        # j=NK-1 first: its groups are pre-opened during the stage prologue.
        w13_tiles = {}
        w24_tiles = {}
        for s in (0, 1):
            for j in [NK - 1] + list(range(NK - 1)):
                w1j = sb_w13.tile([128, NK, 128], bf16, tag="w13")
                nc.sync.dma_start(w1j[:], w1t[s][j * 128:(j + 1) * 128, :]
                                  .rearrange("p (c m) -> p c m", c=NK))
                w3j = sb_w13.tile([128, NK, 128], bf16, tag="w13")
                nc.sync.dma_start(w3j[:], w3t[s][j * 128:(j + 1) * 128, :]
                                  .rearrange("p (c m) -> p c m", c=NK))
                w13_tiles[(s, j)] = (w1j, w3j)
            ch2, ch4 = [], []
            for c in range(NK):
                w2h = sb_ws.tile([128, 1024], bf16, tag="w2h")
                nc.sync.dma_start(w2h[:], w2c[s][c * 128:(c + 1) * 128, :])
                ch2.append(w2h)
                w4h = sb_ws.tile([128, 1024], bf16, tag="w4h")
                nc.sync.dma_start(w4h[:], w4c[s][c * 128:(c + 1) * 128, :])
                ch4.append(w4h)
            w24_tiles[s] = (ch2, ch4)

        # final-linear weights reuse the w13 tag ring (WAR: each lands once
        # the matching stage-1 tile is consumed — early in stage 2)
        wm_tiles = []
        for j2 in range(16):
            wmj = sb_w13.tile([128, NK, 128], bf16, tag="w13")
            nc.sync.dma_start(wmj[:], wmt_ap[j2 * 128:(j2 + 1) * 128, :]
                              .rearrange("p (c m) -> p c m", c=NK))
            wm_tiles.append(wmj)

        # ---- small per-stage constants (scalar queue, all upfront) ----
        stc = {}
        for s in (0, 1):
            sfx = f"_s{s}"
            vecs = sb_st.tile([128, NK, 3], bf16, tag="vecs" + sfx)
            nc.scalar.dma_start(vecs[:], vec[s][:].rearrange("p (c k) -> p c k", c=NK))
            mT = sb_st.tile([128, NK, Q_LEN], bf16, tag="mT" + sfx)
            nc.scalar.dma_start(mT[:], m_t[s][:].rearrange("p (c q) -> p c q", c=NK))
            mN = sb_st.tile([Q_LEN, D], bf16, tag="mN" + sfx)
            nc.scalar.dma_start(mN[:], m_n[s][:])
            mask_i = sb_st.tile([Q_LEN, 1], i32, tag="mask_i" + sfx)
            nc.scalar.dma_start(mask_i[:], msk[s][:])
            stc[s] = (vecs, mT, mN, mask_i)
        bmap_t = sb_st.tile([128, 16], f32, tag="bmap")
        nc.scalar.dma_start(bmap_t[:], bmap_ap[:])

        # ---- hoisted per-stage prologue constants (PE trivial + DVE) ----
        prep = {}
        for s in (0, 1):
            sfx = f"_s{s}"
            vecs, mT, mN, mask_i = stc[s]
            ps_md = ps_m.tile([Q_LEN, 2], f32, tag="ps_m")
            for c in range(NK):
                nc.tensor.matmul(ps_md[:], mT[:, c], vecs[:, c, 1:3],
                                 start=(c == 0), stop=(c == NK - 1))
            maskf = sb_st.tile([Q_LEN, 1], f32, tag="maskf" + sfx)
            nc.vector.tensor_copy(maskf[:], mask_i[:])
            mbias = sb_st.tile([Q_LEN, 1], f32, tag="mbias" + sfx)
            nc.vector.tensor_scalar(mbias[:], maskf[:], NEGBIG, -NEGBIG,
                                    Alu.mult, Alu.add)
            nc.vector.tensor_tensor(mbias[:], mbias[:], ps_md[:, 0:1], Alu.add)
            mst = sb_st.tile([128, NK, Q_LEN + 1], bf16, tag="mst" + sfx)
            nc.vector.tensor_copy(mst[:, :, Q_LEN:Q_LEN + 1], vecs[:, :, 0:1])
            scf = sb_st.tile([128, NK, 1], f32, tag="scf" + sfx)
            nc.vector.tensor_copy(scf[:], vecs[:, :, 2:3])
            for c in range(NK):
                nc.vector.tensor_scalar(mst[:, c, 0:Q_LEN], mT[:, c],
                                        scf[:, c], None, Alu.mult)
            prep[s] = (mbias, mst)

        def run_stage(s, Xt):
            """One BiAttention stage; returns o^T tile [128, NK, R] bf16."""
            sfx = f"_s{s}"
            vecs, mT, mN, mask_i = stc[s]
            mbias, mst = prep[s]

            # ---------- scores for both row-halves (c-outer) ----------
            ps_sc = [ps_att.tile([Q_LEN + 1, 512], f32, tag="ps_sc", name="ps_sc")
                     for _ in range(RH)]
            for c in range(NK):
                for rh in range(RH):
                    sl = slice(rh * 512, (rh + 1) * 512)
                    nc.tensor.matmul(ps_sc[rh][:], mst[:, c], Xt[:, c, sl],
                                     start=(c == 0), stop=(c == NK - 1))

            P_t = [sb_st.tile([Q_LEN, 512], bf16, tag=f"P{rh}" + sfx,
                             name="P_t") for rh in range(RH)]
            vh2 = sb_st.tile([128, RH, NK], f32, tag="vh" + sfx)
            l2col = sb_st.tile([1, 2], f32, tag="l2col" + sfx)
            E_t, eid_t = [], []
            for rh in range(RH):
                sl = slice(rh * 512, (rh + 1) * 512)
                # E = exp(S + membias) (masked -> ~0); eid = exp(input_dot)
                E = sb_rh.tile([Q_LEN, 512], bf16, tag="E")
                eid = sb_rh.tile([1, 512], f32, tag="eid")
                nc.scalar.activation(E[:], ps_sc[rh][0:Q_LEN], AF.Exp,
                                     bias=mbias[:], scale=1.0)
                nc.scalar.activation(eid[:], ps_sc[rh][Q_LEN:Q_LEN + 1], AF.Exp)
                E_t.append(E)
                eid_t.append(eid)
                # softmax normalize: P = E / (column sums of E)
                ps_l1 = ps_m.tile([1, 512], f32, tag="ps_m")
                nc.tensor.matmul(ps_l1[:], ones_q[:], E[:], start=True, stop=True)
                l1r = sb_rh.tile([1, 512], f32, tag="l1r", bufs=1)
                with nc.allow_low_precision(reason="softmax scale approx"):
                    nc.vector.reciprocal_approx_fast(l1r[:], ps_l1[:])
                l1c = sb_rh.tile([1, 512], bf16, tag="l1c")
                nc.vector.tensor_copy(l1c[:], l1r[:])
                ps_rb = ps_m.tile([Q_LEN, 512], f32, tag="ps_m")
                nc.tensor.matmul(ps_rb[:], ones_row[:, 0:Q_LEN], l1c[:],
                                 start=True, stop=True)
                nc.vector.tensor_tensor(P_t[rh][:], E[:], ps_rb[:], Alu.mult)
                mx = sb_rh.tile([Q_LEN, 512], f32, tag="mx", bufs=1)
                nc.gpsimd.partition_all_reduce(mx[:], E_t[rh][:], Q_LEN, RedOp.max)
                e2 = sb_rh.tile([1, 512], f32, tag="e2", bufs=1)
                nc.vector.tensor_tensor(e2[:], mx[0:1], eid_t[rh][:], Alu.mult)
                nc.vector.reduce_sum(l2col[:, rh:rh + 1], e2[:], AX.X)
                e2c = sb_rh.tile([1, 512], bf16, tag="e2c")
                nc.vector.tensor_copy(e2c[:], e2[:])
                ps_eb = ps_m.tile([128, 512], f32, tag="ps_m")
                nc.tensor.matmul(ps_eb[:], ones_row[:], e2c[:], start=True, stop=True)
                ebs = sb_rh.tile([128, 512], bf16, tag="ebs")
                nc.vector.tensor_copy(ebs[:], ps_eb[:])
                scrv = sb_rh.tile([128, 512], bf16, tag="scrv")
                for c in range(NK):
                    nc.vector.scalar_tensor_tensor(
                        scrv[:], Xt[:, c, sl], 1.0, ebs[:],
                        Alu.mult, Alu.mult,
                        accum_out=vh2[:, rh, c:c + 1])

            # ---------- o1^T = mN.T @ P and XO = Xt * o1 ----------
            XO = sb_xo.tile([128, NK, R], bf16, tag="xo")
            for c in range(NK):
                for rh in range(RH):
                    sl = slice(rh * 512, (rh + 1) * 512)
                    ps_o1 = ps_att.tile([128, 512], f32, tag="ps_sc")
                    nc.tensor.matmul(ps_o1[:], mN[:, c * 128:(c + 1) * 128],
                                     P_t[rh][:], start=True, stop=True)
                    nc.vector.tensor_tensor(XO[:, c, sl], Xt[:, c, sl],
                                            ps_o1[:], Alu.mult)

            # pre-open j=NK-1 groups (X half) so PE has work in the prologue
            jpre = NK - 1
            oT = sb_x.tile([128, NK, R], bf16, tag="xt")
            w1p = w13_tiles[(s, jpre)][0]
            ps_pre = []
            for rh in range(RH):
                sl = slice(rh * 512, (rh + 1) * 512)
                ps_ab = ps_o.tile([128, 512], f32, tag="ps_o")
                for c in range(NK):
                    nc.tensor.matmul(ps_ab[:], w1p[:, c], Xt[:, c, sl],
                                     start=(c == 0), stop=False)
                ps_pre.append(ps_ab)


            l2 = sb_st.tile([1, 1], f32, tag="l2" + sfx)
            nc.vector.reduce_sum(l2[:], l2col[:], AX.X)
            vsum = sb_st.tile([128, NK], f32, tag="vsum" + sfx)
            nc.vector.tensor_tensor(vsum[:], vh2[:, 0], vh2[:, 1], Alu.add)
            colsb = sb_st.tile([128, 16], f32, tag="colsb" + sfx)
            nc.vector.memset(colsb[:], 0.0)
            nc.vector.tensor_copy(colsb[:, 0:NK], vsum[:])
            nc.vector.tensor_copy(colsb[0:1, NK:NK + 1], l2[:])
            nc.vector.tensor_copy(colsb[0:1, NK + 1:NK + 2], l2[:])
            cin = dram.tile([128, 16], f32, tag="cin" + sfx)
            cout = dram.tile([128, 16], f32, tag="cout" + sfx)
            nc.gpsimd.dma_start(cin[:], colsb[:])
            nc.gpsimd.collective_compute(
                "AllReduce", Alu.add,
                replica_groups=[[0, 1], [2, 3], [4, 5], [6, 7]],
                ins=[cin[:].opt()], outs=[cout[:].opt()])
            colg = sb_st.tile([128, 16], f32, tag="colg" + sfx)
            nc.gpsimd.dma_start(colg[:], cout[:])

            # ---------- collective-dependent: R = M @ (W2 + t*W4) ----------
            linv = sb_st.tile([1, 2], f32, tag="linv" + sfx)
            with nc.allow_low_precision(reason="weight-two scale approx"):
                nc.vector.reciprocal_approx_fast(linv[:], colg[0:1, NK:NK + 2])
            linc = sb_st.tile([1, 2], bf16, tag="linc" + sfx)
            nc.vector.tensor_copy(linc[:], linv[:])
            ps_tb = ps_m.tile([128, 2], f32, tag="ps_m")
            nc.tensor.matmul(ps_tb[:], ones_row[:], linc[:], start=True, stop=True)
            tvec = sb_st.tile([128, NK], f32, tag="tvec" + sfx)
            nc.vector.tensor_scalar(tvec[:], colg[:, 0:NK], ps_tb[:, 0:1], None, Alu.mult)

            ch2, ch4 = w24_tiles[s]
            ps_r = [ps_m.tile([Q_LEN, 512], f32, tag="ps_m", name="ps_r")
                    for _ in range(2)]
            for c in range(NK):
                w24 = sb_ws.tile([128, 1024], bf16, tag="w24", bufs=3)
                nc.vector.scalar_tensor_tensor(w24[:], ch4[c][:],
                                               tvec[:, c:c + 1], ch2[c][:],
                                               Alu.mult, Alu.add)
                for hf in range(2):
                    slh = slice(hf * 512, (hf + 1) * 512)
                    nc.tensor.matmul(ps_r[hf][:], mT[:, c], w24[:, slh],
                                     start=(c == 0), stop=(c == NK - 1))
            Rsb = sb_st.tile([Q_LEN, D], bf16, tag="Rsb" + sfx)
            for hf in range(2):
                nc.vector.tensor_copy(Rsb[:, hf * 512:(hf + 1) * 512], ps_r[hf][:])

            # ---------- big blocks: oT_j = W1_j@X + W3_j@XO (+ Rsb_j@P) ------
            fold = FOLD[s]

            def close_group(j, rh, ps_ab):
                sl = slice(rh * 512, (rh + 1) * 512)
                for c in range(NK):
                    nc.tensor.matmul(ps_ab[:], w13_tiles[(s, j)][1][:, c],
                                     XO[:, c, sl], start=False,
                                     stop=(j < fold and c == NK - 1))
                if j >= fold:
                    nc.tensor.matmul(ps_ab[:], Rsb[:, j * 128:(j + 1) * 128],
                                     P_t[rh][:], start=False, stop=True)
                nc.vector.tensor_copy(oT[:, j, sl], ps_ab[:])

            for j in range(NK - 1):
                w1j = w13_tiles[(s, j)][0]
                for rh in range(RH):
                    sl = slice(rh * 512, (rh + 1) * 512)
                    ps_ab = ps_o.tile([128, 512], f32, tag="ps_o")
                    for c in range(NK):
                        nc.tensor.matmul(ps_ab[:], w1j[:, c], Xt[:, c, sl],
                                         start=(c == 0), stop=False)
                    close_group(j, rh, ps_ab)
            for rh in range(RH):
                close_group(jpre, rh, ps_pre[rh])

            # separate rank-64 correction for the early-closed groups
            for rh in range(RH):
                sl = slice(rh * 512, (rh + 1) * 512)
                for j in range(fold):
                    ps_c = ps_o.tile([128, 512], f32, tag="ps_o", name="ps_c")
                    nc.tensor.matmul(ps_c[:], Rsb[:, j * 128:(j + 1) * 128],
                                     P_t[rh][:], start=True, stop=True)
                    nc.vector.tensor_tensor(oT[:, j, sl], oT[:, j, sl],
                                            ps_c[:], Alu.add)
            return oT

        o1T = run_stage(0, xt0)
        o2T = run_stage(1, o1T)

        # ---------- final linear (transposed): outT = w_mapT.T @ o2T + b ----
        for j2 in range(16):
            wmj = wm_tiles[j2]
            outsb = sb_o.tile([128, 1024], bf16, tag="outsb")
            for rh in range(RH):
                sl = slice(rh * 512, (rh + 1) * 512)
                ps_f = ps_o.tile([128, 512], f32, tag="ps_o")
                for c in range(NK):
                    nc.tensor.matmul(ps_f[:], wmj[:, c], o2T[:, c, sl],
                                     start=(c == 0), stop=(c == NK - 1))
                nc.vector.tensor_scalar(outsb[:, sl], ps_f[:],
                                        bmap_t[:, j2:j2 + 1], None, Alu.add)
            if j2 == 15:
                for rh in range(RH):
                    sl = slice(rh * 512, (rh + 1) * 512)
                    nc.sync.dma_start(out_ap[j2 * 128:(j2 + 1) * 128, sl],
                                      outsb[:, sl])
            else:
                nc.sync.dma_start(out_ap[j2 * 128:(j2 + 1) * 128, :], outsb[:])

    nc.compile()
    return nc


def _get_nc():
    global _CACHED_NC
    if _CACHED_NC is None:
        _CACHED_NC = _build_nc()
    return _CACHED_NC


def _bf(x):
    return np.ascontiguousarray(np.asarray(x, dtype=np.float32).astype(ml_dtypes.bfloat16))


def _shard_inputs(inputs):
    """Build the 8 per-core input maps (pure layout work, no arithmetic)."""
    x = np.asarray(inputs["ctx_features"], dtype=np.float32)
    q1 = np.asarray(inputs["sub_q1_features"], dtype=np.float32)
    q2 = np.asarray(inputs["sub_q2_features"], dtype=np.float32)
    k1 = np.ascontiguousarray(np.asarray(inputs["sub_q1_attn_mask"], dtype=np.int32))
    k2 = np.ascontiguousarray(np.asarray(inputs["sub_q2_attn_mask"], dtype=np.int32))

    def wblocks(w_out):
        # w_out [D, 4D] -> wb = w_out.T [4D, D]; W_k = wb[kD:(k+1)D]
        wb = np.asarray(w_out, dtype=np.float32).T
        W1, W2, W3, W4 = (wb[k * D:(k + 1) * D] for k in range(4))

        def jmaj(W):  # j-major tiling for the lhsT stream
            return _bf(W.reshape(NK, 128, NK, 128).transpose(2, 1, 0, 3).reshape(D, D))
        return jmaj(W1), _bf(W2), jmaj(W3), _bf(W4)

    w1t1, w2c1, w3t1, w4c1 = wblocks(inputs["w_out1"])
    w1t2, w2c2, w3t2, w4c2 = wblocks(inputs["w_out2"])

    wmT = np.asarray(inputs["w_map"], dtype=np.float32).T  # [D, 2D]
    wmt = _bf(wmT.reshape(NK, 128, 16, 128).transpose(2, 1, 0, 3).reshape(D2, D))
    bmap = np.ascontiguousarray(
        np.asarray(inputs["b_map"], dtype=np.float32).reshape(16, 128).T)

    def ptile_vec(*cols):  # [D] vectors -> [128, NK*k] p-major
        v = np.stack([np.asarray(c, dtype=np.float32) for c in cols], axis=-1)
        k = v.shape[-1]
        return _bf(v.reshape(NK, 128, k).transpose(1, 0, 2).reshape(128, NK * k))

    stage_common = {
        "vec1": ptile_vec(inputs["w_in1"], inputs["w_mem1"], inputs["scale1"]),
        "vec2": ptile_vec(inputs["w_in2"], inputs["w_mem2"], inputs["scale2"]),
        "w1t1": w1t1, "w3t1": w3t1, "w2c1": w2c1, "w4c1": w4c1,
        "w1t2": w1t2, "w3t2": w3t2, "w2c2": w2c2, "w4c2": w4c2,
        "wmt": wmt, "bmap": bmap,
    }

    in_maps = []
    for core in range(N_CORES):
        b, h = divmod(core, 2)
        xT = x[b, h * R:(h + 1) * R, :].T  # [D, R]
        xt_tile = _bf(xT.reshape(NK, 128, R).transpose(1, 0, 2).reshape(128, NK * R))
        m = {}
        for s, q, kk in ((1, q1, k1), (2, q2, k2)):
            mT = q[b].T  # [D, Q]
            m[f"m{s}t"] = _bf(
                mT.reshape(NK, 128, Q_LEN).transpose(1, 0, 2).reshape(128, NK * Q_LEN))
            m[f"m{s}n"] = _bf(q[b])
            m[f"mask{s}"] = np.ascontiguousarray(kk[b].reshape(Q_LEN, 1))
        in_maps.append({"xt": xt_tile, **m, **stage_common})
    return in_maps


def _gather_outputs(results):
    out = np.empty((B, C_LEN, D2), dtype=np.float32)
    for core in range(N_CORES):
        b, h = divmod(core, 2)
        out[b, h * R:(h + 1) * R, :] = results[core]["out"].astype(np.float32).T
    return out


def kernel(**inputs):
    nc = _get_nc()
    in_maps = _shard_inputs(inputs)
    last_err = None
    for _attempt in range(3):
        try:
            res = run_bass_kernel_spmd(nc, in_maps, core_ids=list(range(N_CORES)))
            return _gather_outputs(res.results)
        except Exception as e:  # transient device errors: retry
            last_err = e
    raise last_err


# revision 23
# speedup vs baseline: 1.0872x; 1.0872x over previous
"""Trainium2 Bass kernel for nn_NewModel_42356967473589 (dense_transformer).

Model: two BiAttention blocks + final linear mapping.
  o = BiAttn(ctx, q1) ; o = BiAttn(o, q2) ; out = o @ w_map.T + b_map

Sharding: 8 cores = (batch b in 0..3) x (context half h in 0..1).
Each core owns 1024 context rows of one batch. All compute is row-local
except the softmax-over-context (weight_two); its (sum-exp, weighted-sum)
stats are combined across the pair of cores sharing a batch via a tiny
pairwise AllReduce, overlapped with the large matmuls.

Math restructure (per stage, X = stage input [C,D], M = memory [Q,D]):
  out = X@W1 + o1@W2 + (X*o1)@W3 + (t*o1)@W4      (W_k = w_out[:, kD:(k+1)D].T)
  o1 = P@M (rank Q=64), t broadcast over rows =>
  o1@W2 + (t*o1)@W4 = P @ (M @ (W2 + t*W4))        (rank-64 path)
All tensors are kept transposed on-chip ([D on partitions, rows free]) and
in bf16 (rel tolerance is 2e-2; bf16 end-to-end lands ~5e-3), which halves
HBM traffic, halves LDWEIGHTS (fast weight load), and halves DVE time.
PSUM accumulation stays fp32.

Engine-queue discipline (head-of-line blocking is real):
  sync   = all bulk weight/input DMA (xt, W1/W3, W2/W4, w_map), out stores
  scalar = tiny consts, exp activations, PSUM->SBUF copies, bias adds
  gpsimd = partition max-reduce, collective staging + AllReduce (nothing else)
"""

import numpy as np
import ml_dtypes

import concourse.bacc as bacc
import concourse.tile as tile
from concourse import mybir
from concourse.bass_utils import run_bass_kernel_spmd
from contextlib import ExitStack
import bass_rust

f32 = mybir.dt.float32
bf16 = mybir.dt.bfloat16
i32 = mybir.dt.int32
Alu = mybir.AluOpType
AF = bass_rust.ActivationFunctionType
AX = bass_rust.AxisListType
RedOp = bass_rust.ReduceOp

B, C_LEN, Q_LEN, D = 4, 2048, 64, 1024
N_CORES = 8
R = C_LEN // 2          # rows per core
NK = D // 128           # contraction chunks
RH = R // 512           # row halves (moving-dim tiles)
D2 = 2 * D
NEGBIG = 10000.0
# correction fold threshold per stage: groups j >= FOLD close after the
# collective result lands anyway, so the rank-64 term accumulates in-group;
# earlier groups close immediately and get a separate correction pass.
FOLD = {0: 5, 1: 3}

_CACHED_NC = None


def _build_nc():
    nc = bacc.Bacc("TRN2", target_bir_lowering=False, debug=False,
                   num_devices=N_CORES)

    # ---- per-core DRAM I/O (host pre-tiled layouts, see _shard_inputs) ----
    xt_ap = nc.dram_tensor("xt", [128, NK * R], bf16, kind="ExternalInput").ap()
    m_t = [nc.dram_tensor(f"m{s}t", [128, NK * Q_LEN], bf16, kind="ExternalInput").ap() for s in (1, 2)]
    m_n = [nc.dram_tensor(f"m{s}n", [Q_LEN, D], bf16, kind="ExternalInput").ap() for s in (1, 2)]
    vec = [nc.dram_tensor(f"vec{s}", [128, NK * 3], bf16, kind="ExternalInput").ap() for s in (1, 2)]
    msk = [nc.dram_tensor(f"mask{s}", [Q_LEN, 1], i32, kind="ExternalInput").ap() for s in (1, 2)]
    w1t = [nc.dram_tensor(f"w1t{s}", [D, D], bf16, kind="ExternalInput").ap() for s in (1, 2)]
    w3t = [nc.dram_tensor(f"w3t{s}", [D, D], bf16, kind="ExternalInput").ap() for s in (1, 2)]
    w2c = [nc.dram_tensor(f"w2c{s}", [D, D], bf16, kind="ExternalInput").ap() for s in (1, 2)]
    w4c = [nc.dram_tensor(f"w4c{s}", [D, D], bf16, kind="ExternalInput").ap() for s in (1, 2)]
    wmt_ap = nc.dram_tensor("wmt", [D2, D], bf16, kind="ExternalInput").ap()
    bmap_ap = nc.dram_tensor("bmap", [128, 16], f32, kind="ExternalInput").ap()
    out_ap = nc.dram_tensor("out", [D2, R], bf16, kind="ExternalOutput").ap()

    with tile.TileContext(nc) as tc, ExitStack() as ctx:
        sb_x = ctx.enter_context(tc.tile_pool(name="sb_x", bufs=2))
        sb_xo = ctx.enter_context(tc.tile_pool(name="sb_xo", bufs=1))
        sb_w13 = ctx.enter_context(tc.tile_pool(name="sb_w13", bufs=32))
        sb_ws = ctx.enter_context(tc.tile_pool(name="sb_ws", bufs=8))
        sb_o = ctx.enter_context(tc.tile_pool(name="sb_o", bufs=3))
        sb_rh = ctx.enter_context(tc.tile_pool(name="sb_rh", bufs=2))
        sb_st = ctx.enter_context(tc.tile_pool(name="sb_st", bufs=1))
        ps_o = ctx.enter_context(tc.tile_pool(name="ps_o", bufs=4, space="PSUM"))
        ps_att = ctx.enter_context(tc.tile_pool(name="ps_att", bufs=2, space="PSUM"))
        ps_m = ctx.enter_context(tc.tile_pool(name="ps_m", bufs=2, space="PSUM"))
        dram = ctx.enter_context(tc.tile_pool(name="dram", bufs=2, space="DRAM"))

        # ---- constants ----
        ones_row = sb_st.tile([1, 128], bf16, tag="ones_row")
        nc.vector.memset(ones_row[:], 1.0)
        ones_q = sb_st.tile([Q_LEN, 1], bf16, tag="ones_q")
        nc.vector.memset(ones_q[:], 1.0)

        # ---- PE warmup: ~3.5us of dummy matmuls so the HAM clock-gate
        # opens to 8/8 while the input DMA is still in flight ----
        if True:  # dummy PE warmup: opens the HAM clock gate early
            dum = sb_st.tile([1, 512], bf16, tag="dum")
            nc.vector.memset(dum[:], 0.0)
            ps_w = ps_m.tile([128, 512], f32, tag="ps_m")
            for k in range(18):
                nc.tensor.matmul(ps_w[:], ones_row[:], dum[:],
                                 start=(k == 0), stop=(k == 17))

        # ---- warmup collective: absorbs core start-skew so the stage-1
        # stats AllReduce is fast; gpsimd has nothing urgent until ~+12us ----
        wsb = sb_st.tile([1, 4], f32, tag="wsb")
        nc.vector.memset(wsb[:], 0.0)
        win = dram.tile([1, 4], f32, tag="win")
        wout = dram.tile([1, 4], f32, tag="wout")
        nc.gpsimd.dma_start(win[:], wsb[:])
        nc.gpsimd.collective_compute(
            "AllReduce", Alu.add,
            replica_groups=[[0, 1], [2, 3], [4, 5], [6, 7]],
            ins=[win[:].opt()], outs=[wout[:].opt()])

        # ---- input X^T (sync queue, first) ----
        xt0 = sb_x.tile([128, NK, R], bf16, tag="xt")
        nc.sync.dma_start(xt0[:], xt_ap[:].rearrange("p (c r) -> p c r", c=NK))

        # ---- bulk weight prefetch (sync queue, in consumption order) ----
        # j=NK-1 first: its groups are pre-opened during the stage prologue.
        w13_tiles = {}
        w24_tiles = {}
        for s in (0, 1):
            for j in [NK - 1] + list(range(NK - 1)):
                w1j = sb_w13.tile([128, NK, 128], bf16, tag="w13")
                nc.sync.dma_start(w1j[:], w1t[s][j * 128:(j + 1) * 128, :]
                                  .rearrange("p (c m) -> p c m", c=NK))
                w3j = sb_w13.tile([128, NK, 128], bf16, tag="w13")
                nc.sync.dma_start(w3j[:], w3t[s][j * 128:(j + 1) * 128, :]
                                  .rearrange("p (c m) -> p c m", c=NK))
                w13_tiles[(s, j)] = (w1j, w3j)
            ch2, ch4 = [], []
            for c in range(NK):
                w2h = sb_ws.tile([128, 1024], bf16, tag="w2h")
                nc.sync.dma_start(w2h[:], w2c[s][c * 128:(c + 1) * 128, :])
                ch2.append(w2h)
                w4h = sb_ws.tile([128, 1024], bf16, tag="w4h")
                nc.sync.dma_start(w4h[:], w4c[s][c * 128:(c + 1) * 128, :])
                ch4.append(w4h)
            w24_tiles[s] = (ch2, ch4)

        # final-linear weights reuse the w13 tag ring (WAR: each lands once
        # the matching stage-1 tile is consumed — early in stage 2)
        wm_tiles = []
        for j2 in range(16):
            wmj = sb_w13.tile([128, NK, 128], bf16, tag="w13")
            nc.sync.dma_start(wmj[:], wmt_ap[j2 * 128:(j2 + 1) * 128, :]
                              .rearrange("p (c m) -> p c m", c=NK))
            wm_tiles.append(wmj)

        # ---- small per-stage constants (scalar queue, all upfront) ----
        stc = {}
        for s in (0, 1):
            sfx = f"_s{s}"
            vecs = sb_st.tile([128, NK, 3], bf16, tag="vecs" + sfx)
            nc.scalar.dma_start(vecs[:], vec[s][:].rearrange("p (c k) -> p c k", c=NK))
            mT = sb_st.tile([128, NK, Q_LEN], bf16, tag="mT" + sfx)
            nc.scalar.dma_start(mT[:], m_t[s][:].rearrange("p (c q) -> p c q", c=NK))
            mN = sb_st.tile([Q_LEN, D], bf16, tag="mN" + sfx)
            nc.scalar.dma_start(mN[:], m_n[s][:])
            mask_i = sb_st.tile([Q_LEN, 1], i32, tag="mask_i" + sfx)
            nc.scalar.dma_start(mask_i[:], msk[s][:])
            stc[s] = (vecs, mT, mN, mask_i)
        bmap_t = sb_st.tile([128, 16], f32, tag="bmap")
        nc.scalar.dma_start(bmap_t[:], bmap_ap[:])

        # ---- hoisted per-stage prologue constants (PE trivial + DVE) ----
        prep = {}
        for s in (0, 1):
            sfx = f"_s{s}"
            vecs, mT, mN, mask_i = stc[s]
            ps_md = ps_m.tile([Q_LEN, 2], f32, tag="ps_m")
            for c in range(NK):
                nc.tensor.matmul(ps_md[:], mT[:, c], vecs[:, c, 1:3],
                                 start=(c == 0), stop=(c == NK - 1))
            maskf = sb_st.tile([Q_LEN, 1], f32, tag="maskf" + sfx)
            nc.vector.tensor_copy(maskf[:], mask_i[:])
            mbias = sb_st.tile([Q_LEN, 1], f32, tag="mbias" + sfx)
            nc.vector.tensor_scalar(mbias[:], maskf[:], NEGBIG, -NEGBIG,
                                    Alu.mult, Alu.add)
            nc.vector.tensor_tensor(mbias[:], mbias[:], ps_md[:, 0:1], Alu.add)
            mst = sb_st.tile([128, NK, Q_LEN + 1], bf16, tag="mst" + sfx)
            nc.vector.tensor_copy(mst[:, :, Q_LEN:Q_LEN + 1], vecs[:, :, 0:1])
            scf = sb_st.tile([128, NK, 1], f32, tag="scf" + sfx)
            nc.vector.tensor_copy(scf[:], vecs[:, :, 2:3])
            for c in range(NK):
                nc.vector.tensor_scalar(mst[:, c, 0:Q_LEN], mT[:, c],
                                        scf[:, c], None, Alu.mult)
            prep[s] = (mbias, mst)

        def run_stage(s, Xt):
            """One BiAttention stage; returns o^T tile [128, NK, R] bf16."""
            sfx = f"_s{s}"
            vecs, mT, mN, mask_i = stc[s]
            mbias, mst = prep[s]

            # ---------- scores for both row-halves (c-outer) ----------
            ps_sc = [ps_att.tile([Q_LEN + 1, 512], f32, tag="ps_sc", name="ps_sc")
                     for _ in range(RH)]
            for c in range(NK):
                for rh in range(RH):
                    sl = slice(rh * 512, (rh + 1) * 512)
                    nc.tensor.matmul(ps_sc[rh][:], mst[:, c], Xt[:, c, sl],
                                     start=(c == 0), stop=(c == NK - 1))

            P_t = [sb_st.tile([Q_LEN, 512], bf16, tag=f"P{rh}" + sfx,
                             name="P_t") for rh in range(RH)]
            E_t, eid_t = [], []
            for rh in range(RH):
                # E = exp(S + membias) (masked -> ~0); eid = exp(input_dot)
                E = sb_rh.tile([Q_LEN, 512], bf16, tag="E")
                eid = sb_rh.tile([1, 512], f32, tag="eid")
                nc.scalar.activation(E[:], ps_sc[rh][0:Q_LEN], AF.Exp,
                                     bias=mbias[:], scale=1.0)
                nc.scalar.activation(eid[:], ps_sc[rh][Q_LEN:Q_LEN + 1], AF.Exp)
                E_t.append(E)
                eid_t.append(eid)
                # softmax normalize: P = E / (column sums of E)
                ps_l1 = ps_m.tile([1, 512], f32, tag="ps_m")
                nc.tensor.matmul(ps_l1[:], ones_q[:], E[:], start=True, stop=True)
                l1r = sb_rh.tile([1, 512], f32, tag="l1r", bufs=1)
                with nc.allow_low_precision(reason="softmax scale approx"):
                    nc.vector.reciprocal_approx_fast(l1r[:], ps_l1[:])
                l1c = sb_rh.tile([1, 512], bf16, tag="l1c")
                nc.vector.tensor_copy(l1c[:], l1r[:])
                ps_rb = ps_m.tile([Q_LEN, 512], f32, tag="ps_m")
                nc.tensor.matmul(ps_rb[:], ones_row[:, 0:Q_LEN], l1c[:],
                                 start=True, stop=True)
                nc.vector.tensor_tensor(P_t[rh][:], E[:], ps_rb[:], Alu.mult)

            # ---------- o1^T = mN.T @ P and XO = Xt * o1 ----------
            XO = sb_xo.tile([128, NK, R], bf16, tag="xo")
            for c in range(NK):
                for rh in range(RH):
                    sl = slice(rh * 512, (rh + 1) * 512)
                    ps_o1 = ps_att.tile([128, 512], f32, tag="ps_sc")
                    nc.tensor.matmul(ps_o1[:], mN[:, c * 128:(c + 1) * 128],
                                     P_t[rh][:], start=True, stop=True)
                    nc.vector.tensor_tensor(XO[:, c, sl], Xt[:, c, sl],
                                            ps_o1[:], Alu.mult)

            # pre-open j=NK-1 groups (X half) so PE has work in the prologue
            jpre = NK - 1
            oT = sb_x.tile([128, NK, R], bf16, tag="xt")
            w1p = w13_tiles[(s, jpre)][0]
            ps_pre = []
            for rh in range(RH):
                sl = slice(rh * 512, (rh + 1) * 512)
                ps_ab = ps_o.tile([128, 512], f32, tag="ps_o")
                for c in range(NK):
                    nc.tensor.matmul(ps_ab[:], w1p[:, c], Xt[:, c, sl],
                                     start=(c == 0), stop=False)
                ps_pre.append(ps_ab)

            # ---------- weight_two stats -> pairwise AllReduce (tiny) ----------
            vh2 = sb_st.tile([128, RH, NK], f32, tag="vh" + sfx)
            l2col = sb_st.tile([1, 2], f32, tag="l2col" + sfx)
            for rh in range(RH):
                sl = slice(rh * 512, (rh + 1) * 512)
                mx = sb_rh.tile([Q_LEN, 512], f32, tag="mx", bufs=1)
                nc.gpsimd.partition_all_reduce(mx[:], E_t[rh][:], Q_LEN, RedOp.max)
                e2 = sb_rh.tile([1, 512], f32, tag="e2", bufs=1)
                nc.vector.tensor_tensor(e2[:], mx[0:1], eid_t[rh][:], Alu.mult)
                nc.vector.reduce_sum(l2col[:, rh:rh + 1], e2[:], AX.X)
                e2c = sb_rh.tile([1, 512], bf16, tag="e2c")
                nc.vector.tensor_copy(e2c[:], e2[:])
                ps_eb = ps_m.tile([128, 512], f32, tag="ps_m")
                nc.tensor.matmul(ps_eb[:], ones_row[:], e2c[:], start=True, stop=True)
                ebs = sb_rh.tile([128, 512], bf16, tag="ebs")
                nc.vector.tensor_copy(ebs[:], ps_eb[:])
                scrv = sb_rh.tile([128, 512], bf16, tag="scrv")
                for c in range(NK):
                    nc.vector.scalar_tensor_tensor(
                        scrv[:], Xt[:, c, sl], 1.0, ebs[:],
                        Alu.mult, Alu.mult,
                        accum_out=vh2[:, rh, c:c + 1])

            l2 = sb_st.tile([1, 1], f32, tag="l2" + sfx)
            nc.vector.reduce_sum(l2[:], l2col[:], AX.X)
            vsum = sb_st.tile([128, NK], f32, tag="vsum" + sfx)
            nc.vector.tensor_tensor(vsum[:], vh2[:, 0], vh2[:, 1], Alu.add)
            colsb = sb_st.tile([128, 16], f32, tag="colsb" + sfx)
            nc.vector.memset(colsb[:], 0.0)
            nc.vector.tensor_copy(colsb[:, 0:NK], vsum[:])
            nc.vector.tensor_copy(colsb[0:1, NK:NK + 1], l2[:])
            nc.vector.tensor_copy(colsb[0:1, NK + 1:NK + 2], l2[:])
            cin = dram.tile([128, 16], f32, tag="cin" + sfx)
            cout = dram.tile([128, 16], f32, tag="cout" + sfx)
            nc.gpsimd.dma_start(cin[:], colsb[:])
            nc.gpsimd.collective_compute(
                "AllReduce", Alu.add,
                replica_groups=[[0, 1], [2, 3], [4, 5], [6, 7]],
                ins=[cin[:].opt()], outs=[cout[:].opt()])
            colg = sb_st.tile([128, 16], f32, tag="colg" + sfx)
            nc.gpsimd.dma_start(colg[:], cout[:])

            # ---------- collective-dependent: R = M @ (W2 + t*W4) ----------
            linv = sb_st.tile([1, 2], f32, tag="linv" + sfx)
            with nc.allow_low_precision(reason="weight-two scale approx"):
                nc.vector.reciprocal_approx_fast(linv[:], colg[0:1, NK:NK + 2])
            linc = sb_st.tile([1, 2], bf16, tag="linc" + sfx)
            nc.vector.tensor_copy(linc[:], linv[:])
            ps_tb = ps_m.tile([128, 2], f32, tag="ps_m")
            nc.tensor.matmul(ps_tb[:], ones_row[:], linc[:], start=True, stop=True)
            tvec = sb_st.tile([128, NK], f32, tag="tvec" + sfx)
            nc.vector.tensor_scalar(tvec[:], colg[:, 0:NK], ps_tb[:, 0:1], None, Alu.mult)

            ch2, ch4 = w24_tiles[s]
            ps_r = [ps_m.tile([Q_LEN, 512], f32, tag="ps_m", name="ps_r")
                    for _ in range(2)]
            for c in range(NK):
                w24 = sb_ws.tile([128, 1024], bf16, tag="w24", bufs=3)
                nc.vector.scalar_tensor_tensor(w24[:], ch4[c][:],
                                               tvec[:, c:c + 1], ch2[c][:],
                                               Alu.mult, Alu.add)
                for hf in range(2):
                    slh = slice(hf * 512, (hf + 1) * 512)
                    nc.tensor.matmul(ps_r[hf][:], mT[:, c], w24[:, slh],
                                     start=(c == 0), stop=(c == NK - 1))
            Rsb = sb_st.tile([Q_LEN, D], bf16, tag="Rsb" + sfx)
            for hf in range(2):
                nc.vector.tensor_copy(Rsb[:, hf * 512:(hf + 1) * 512], ps_r[hf][:])

            # ---------- big blocks: oT_j = W1_j@X + W3_j@XO (+ Rsb_j@P) ------
            fold = FOLD[s]

            def close_group(j, rh, ps_ab):
                sl = slice(rh * 512, (rh + 1) * 512)
                for c in range(NK):
                    nc.tensor.matmul(ps_ab[:], w13_tiles[(s, j)][1][:, c],
                                     XO[:, c, sl], start=False,
                                     stop=(j < fold and c == NK - 1))
                if j >= fold:
                    nc.tensor.matmul(ps_ab[:], Rsb[:, j * 128:(j + 1) * 128],
                                     P_t[rh][:], start=False, stop=True)
                nc.vector.tensor_copy(oT[:, j, sl], ps_ab[:])

            for j in range(NK - 1):
                w1j = w13_tiles[(s, j)][0]
                for rh in range(RH):
                    sl = slice(rh * 512, (rh + 1) * 512)
                    ps_ab = ps_o.tile([128, 512], f32, tag="ps_o")
                    for c in range(NK):
                        nc.tensor.matmul(ps_ab[:], w1j[:, c], Xt[:, c, sl],
                                         start=(c == 0), stop=False)
                    close_group(j, rh, ps_ab)
            for rh in range(RH):
                close_group(jpre, rh, ps_pre[rh])

            # separate rank-64 correction for the early-closed groups
            for rh in range(RH):
                sl = slice(rh * 512, (rh + 1) * 512)
                for j in range(fold):
                    ps_c = ps_o.tile([128, 512], f32, tag="ps_o", name="ps_c")
                    nc.tensor.matmul(ps_c[:], Rsb[:, j * 128:(j + 1) * 128],
                                     P_t[rh][:], start=True, stop=True)
                    nc.vector.tensor_tensor(oT[:, j, sl], oT[:, j, sl],
                                            ps_c[:], Alu.add)
            return oT

        o1T = run_stage(0, xt0)
        o2T = run_stage(1, o1T)

        # ---------- final linear (transposed): outT = w_mapT.T @ o2T + b ----
        for j2 in range(16):
            wmj = wm_tiles[j2]
            outsb = sb_o.tile([128, 1024], bf16, tag="outsb")
            for rh in range(RH):
                sl = slice(rh * 512, (rh + 1) * 512)
                ps_f = ps_o.tile([128, 512], f32, tag="ps_o")
                for c in range(NK):
                    nc.tensor.matmul(ps_f[:], wmj[:, c], o2T[:, c, sl],
                                     start=(c == 0), stop=(c == NK - 1))
                nc.vector.tensor_scalar(outsb[:, sl], ps_f[:],
                                        bmap_t[:, j2:j2 + 1], None, Alu.add)
            nc.sync.dma_start(out_ap[j2 * 128:(j2 + 1) * 128, :], outsb[:])

    nc.compile()
    return nc


def _get_nc():
    global _CACHED_NC
    if _CACHED_NC is None:
        _CACHED_NC = _build_nc()
    return _CACHED_NC


def _bf(x):
    return np.ascontiguousarray(np.asarray(x, dtype=np.float32).astype(ml_dtypes.bfloat16))


def _shard_inputs(inputs):
    """Build the 8 per-core input maps (pure layout work, no arithmetic)."""
    x = np.asarray(inputs["ctx_features"], dtype=np.float32)
    q1 = np.asarray(inputs["sub_q1_features"], dtype=np.float32)
    q2 = np.asarray(inputs["sub_q2_features"], dtype=np.float32)
    k1 = np.ascontiguousarray(np.asarray(inputs["sub_q1_attn_mask"], dtype=np.int32))
    k2 = np.ascontiguousarray(np.asarray(inputs["sub_q2_attn_mask"], dtype=np.int32))

    def wblocks(w_out):
        # w_out [D, 4D] -> wb = w_out.T [4D, D]; W_k = wb[kD:(k+1)D]
        wb = np.asarray(w_out, dtype=np.float32).T
        W1, W2, W3, W4 = (wb[k * D:(k + 1) * D] for k in range(4))

        def jmaj(W):  # j-major tiling for the lhsT stream
            return _bf(W.reshape(NK, 128, NK, 128).transpose(2, 1, 0, 3).reshape(D, D))
        return jmaj(W1), _bf(W2), jmaj(W3), _bf(W4)

    w1t1, w2c1, w3t1, w4c1 = wblocks(inputs["w_out1"])
    w1t2, w2c2, w3t2, w4c2 = wblocks(inputs["w_out2"])

    wmT = np.asarray(inputs["w_map"], dtype=np.float32).T  # [D, 2D]
    wmt = _bf(wmT.reshape(NK, 128, 16, 128).transpose(2, 1, 0, 3).reshape(D2, D))
    bmap = np.ascontiguousarray(
        np.asarray(inputs["b_map"], dtype=np.float32).reshape(16, 128).T)

    def ptile_vec(*cols):  # [D] vectors -> [128, NK*k] p-major
        v = np.stack([np.asarray(c, dtype=np.float32) for c in cols], axis=-1)
        k = v.shape[-1]
        return _bf(v.reshape(NK, 128, k).transpose(1, 0, 2).reshape(128, NK * k))

    stage_common = {
        "vec1": ptile_vec(inputs["w_in1"], inputs["w_mem1"], inputs["scale1"]),
        "vec2": ptile_vec(inputs["w_in2"], inputs["w_mem2"], inputs["scale2"]),
        "w1t1": w1t1, "w3t1": w3t1, "w2c1": w2c1, "w4c1": w4c1,
        "w1t2": w1t2, "w3t2": w3t2, "w2c2": w2c2, "w4c2": w4c2,
        "wmt": wmt, "bmap": bmap,
    }

    in_maps = []
    for core in range(N_CORES):
        b, h = divmod(core, 2)
        xT = x[b, h * R:(h + 1) * R, :].T  # [D, R]
        xt_tile = _bf(xT.reshape(NK, 128, R).transpose(1, 0, 2).reshape(128, NK * R))
        m = {}
        for s, q, kk in ((1, q1, k1), (2, q2, k2)):
            mT = q[b].T  # [D, Q]
            m[f"m{s}t"] = _bf(
                mT.reshape(NK, 128, Q_LEN).transpose(1, 0, 2).reshape(128, NK * Q_LEN))
            m[f"m{s}n"] = _bf(q[b])
            m[f"mask{s}"] = np.ascontiguousarray(kk[b].reshape(Q_LEN, 1))
        in_maps.append({"xt": xt_tile, **m, **stage_common})
    return in_maps


def _gather_outputs(results):
    out = np.empty((B, C_LEN, D2), dtype=np.float32)
    for core in range(N_CORES):
        b, h = divmod(core, 2)
        out[b, h * R:(h + 1) * R, :] = results[core]["out"].astype(np.float32).T
    return out


def kernel(**inputs):
    nc = _get_nc()
    in_maps = _shard_inputs(inputs)
    last_err = None
    for _attempt in range(3):
        try:
            res = run_bass_kernel_spmd(nc, in_maps, core_ids=list(range(N_CORES)))
            return _gather_outputs(res.results)
        except Exception as e:  # transient device errors: retry
            last_err = e
    raise last_err
